# revision 42
# baseline (speedup 1.0000x reference)
"""Trainium2 Bass kernel for a 12-layer BERT encoder forward pass.

Strategy: data-parallel over the batch across 8 NeuronCores (2 sequences
each), no collectives. Activations are kept feature-major on-chip in fp16;
matmul weights are fp16 and host-pretransposed into SBUF-ready block layout
so every weight is DMA'd exactly once per layer with fully contiguous
descriptors. Attention uses a transposed-scores layout with a ones-column
appended to V so the softmax denominator falls out of the AV matmul
(exp-values and V are f32r to keep exp in f32 range at full PE speed).
The V bias is folded into the output-projection bias on the host.
PSUM accumulation, softmax and LayerNorm statistics stay f32.
kernel(**inputs) takes the full inputs and returns the full [16,512,768]
f32 output.
"""
import sys
for _p in ('/opt/trn_rl_repo', '/root/.axon_site/_ro/trn_rl_repo'):
    if _p not in sys.path:
        sys.path.append(_p)
import numpy as np
from contextlib import ExitStack

import concourse.bass as bass
from concourse import bacc
import concourse.mybir as mybir
import concourse.tile as tile
from concourse.masks import make_identity
from concourse import tile_utils

# allow using the full usable SBUF (stale default is 192KB/partition)
tile_utils.max_sbuf_usage = 208 * 1024

f32 = mybir.dt.float32
f32r = mybir.dt.float32r
f16 = mybir.dt.float16
i32 = mybir.dt.int32
AF = mybir.ActivationFunctionType
ALU = mybir.AluOpType

P = 128
D = 768
KC = 6          # D / P
H = 12
HD = 64         # head dim
F = 3072
FC = 24         # F / P
S = 512
N = 1024        # tokens per core (2 seqs)
NT = 8          # N / P
EPS = 1e-12

# params column layout: 8 blocks of KC cols + FC cols for bf1
_PC = {"bq": 0, "bk": 6, "bo": 12, "bf2": 18,
       "l1w": 24, "l1b": 30, "l2w": 36, "l2b": 42, "bf1": 48}
PARAM_COLS = 72


class Ctx:
    pass


def build_nc(L=12, use_f32r=True, gelu_sim=False, reps=1):
    g = Ctx()
    nc = bacc.Bacc("TRN2", num_devices=8, dynamic_dma_scratch_size=4096)
    g.nc = nc
    g.act_fn = AF.Tanh if gelu_sim else AF.Gelu

    # ---- DRAM inputs ----
    g.ids = nc.dram_tensor("ids", [N, 1], i32, kind="ExternalInput")
    g.word_emb = nc.dram_tensor("word_emb", [30522, D], f16, kind="ExternalInput")
    g.pos_type = nc.dram_tensor("pos_type", [S, D], f16, kind="ExternalInput")
    g.emb_w = nc.dram_tensor("emb_w", [1, D], f32, kind="ExternalInput")
    g.emb_b = nc.dram_tensor("emb_b", [1, D], f32, kind="ExternalInput")
    g.WqT = nc.dram_tensor("WqT", [L, KC, P, D], f16, kind="ExternalInput")
    g.WkT = nc.dram_tensor("WkT", [L, KC, P, D], f16, kind="ExternalInput")
    g.WvT = nc.dram_tensor("WvT", [L, P, KC * D], f16, kind="ExternalInput")
    g.WoT = nc.dram_tensor("WoT", [L, KC, P, D], f16, kind="ExternalInput")
    g.Wf1T = nc.dram_tensor("Wf1T", [L, FC, P, D], f16, kind="ExternalInput")
    g.Wf2T = nc.dram_tensor("Wf2T", [L, KC, P, F], f16, kind="ExternalInput")
    g.params = nc.dram_tensor("params", [L, P, PARAM_COLS], f32, kind="ExternalInput")
    g.out_fm = nc.dram_tensor("out_fm", [KC, P, N], f16, kind="ExternalOutput")

    with TileContextPools(g) as g:
        if reps > 1:
            with g.tc.For_i(0, reps, 1):
                _emit(g, L)
        else:
            _emit(g, L)

    nc.finalize()
    return nc


class TileContextPools:
    def __init__(self, g):
        self.g = g

    def __enter__(self):
        g = self.g
        self.stack = ExitStack()
        tc = self.stack.enter_context(tile.TileContext(g.nc))
        ep = self.stack.enter_context
        g.tc = tc
        g.act = ep(tc.tile_pool(name="act", bufs=5))      # fp16 [P,KC,N] = 12KB
        g.ffp = ep(tc.tile_pool(name="ffp", bufs=1))      # ffT fp16 48KB / htok f32
        g.vp = ep(tc.tile_pool(name="vp", bufs=1))        # v f32r 24.4KB
        g.wvp = ep(tc.tile_pool(name="wvp", bufs=2))      # Wv fp16 9KB
        g.wp = ep(tc.tile_pool(name="wp", bufs=4))        # weight blocks fp16 1.5KB
        g.w2p = ep(tc.tile_pool(name="w2p", bufs=2))      # Wf2 blocks fp16 6KB
        g.qmp = ep(tc.tile_pool(name="qmp", bufs=3))      # qm/kblk fp16 2KB
        g.sqp = ep(tc.tile_pool(name="sqp", bufs=2))      # LN x^2 fp16 1KB
        g.expp = ep(tc.tile_pool(name="expp", bufs=5))    # exp f32r 2KB
        g.dvp = ep(tc.tile_pool(name="dvp", bufs=2))      # dinv bcast f32
        g.bb = ep(tc.tile_pool(name="bb", bufs=8))        # LN A/B bcast f16 1KB
        g.rowsp = ep(tc.tile_pool(name="rows", bufs=1))
        g.rowp = ep(tc.tile_pool(name="rowp", bufs=1))
        g.singles = ep(tc.tile_pool(name="singles", bufs=1))
        g.small = ep(tc.tile_pool(name="small", bufs=2))
        g.biasp = ep(tc.tile_pool(name="bias", bufs=2))
        g.pp = ep(tc.tile_pool(name="pp", bufs=4, space="PSUM"))
        g.scp = ep(tc.tile_pool(name="scp", bufs=2, space="PSUM"))
        g.avp = ep(tc.tile_pool(name="avp", bufs=2, space="PSUM"))
        return g

    def __exit__(self, *a):
        return self.stack.__exit__(*a)


def _emit(g, L):
    nc = g.nc
    g.ident = g.singles.tile([P, P], f32, tag="ident")
    make_identity(nc, g.ident[:])
    g.ones = g.singles.tile([P, 1], f16, tag="ones")
    nc.vector.memset(g.ones[:], 1.0)
    g.epsT = g.singles.tile([P, 1], f32, tag="eps")
    nc.vector.memset(g.epsT[:], EPS)
    g.onesr = g.singles.tile([P, 1], f32, tag="onesr")
    nc.vector.memset(g.onesr[:], 1.0)

    hT = _embedding(g)
    for l in range(L):
        hT = _layer(g, l, hT)
    for k in range(KC):
        nc.sync.dma_start(out=g.out_fm[k], in_=hT[:, k, :])


def _embedding(g):
    nc = g.nc
    lnw_b = g.wvp.tile([P, D], f32, tag="wv", name="emb_lnw")
    lnb_b = g.wvp.tile([P, D], f32, tag="wv", name="emb_lnb")
    embwb = g.rowp.tile([1, 2, D], f32, tag="row")
    nc.sync.dma_start(out=embwb[:, 0, :], in_=g.emb_w[:])
    nc.gpsimd.partition_broadcast(lnw_b[:], embwb[:, 0, :])
    nc.sync.dma_start(out=embwb[:, 1, :], in_=g.emb_b[:])
    nc.gpsimd.partition_broadcast(lnb_b[:], embwb[:, 1, :])

    htok = g.ffp.tile([P, NT, D], f32, tag="ff")
    hT = g.act.tile([P, KC, N], f16, tag="act")
    for tt in range(NT):
        _embed_tile(g, htok, tt, lnw_b, lnb_b)
        for k in range(KC):
            ps = g.pp.tile([P, 512], f32, tag="pp")
            nc.tensor.transpose(ps[:, 0:P], htok[:, tt, k * P:(k + 1) * P], g.ident[:])
            nc.vector.tensor_copy(hT[:, k, tt * P:(tt + 1) * P], ps[:, 0:P])
    return hT


def _embed_tile(g, htok, tt, lnw_b, lnb_b):
    nc = g.nc
    idx = g.small.tile([P, 1], i32, tag="idx")
    nc.sync.dma_start(out=idx[:], in_=g.ids[tt * P:(tt + 1) * P, :])
    gt = g.wp.tile([P, D], f16, tag="w")
    nc.gpsimd.indirect_dma_start(
        out=gt[:], out_offset=None, in_=g.word_emb[:],
        in_offset=bass.IndirectOffsetOnAxis(ap=idx[:, :1], axis=0),
    )
    pt = g.wp.tile([P, D], f16, tag="w")
    nc.sync.dma_start(out=pt[:], in_=g.pos_type[(tt % 4) * P:(tt % 4 + 1) * P, :])
    nc.vector.tensor_add(htok[:, tt, :], gt[:], pt[:])
    xr = htok[:, tt, :].rearrange("p (s f) -> p s f", f=256)
    stats = g.small.tile([P, 3, 6], f32, tag="bnst")
    for sgi in range(3):
        nc.vector.bn_stats(out=stats[:, sgi, :], in_=xr[:, sgi, :])
    mv = g.small.tile([P, 2], f32, tag="bnmv")
    nc.vector.bn_aggr(out=mv[:], in_=stats[:])
    sd = g.small.tile([P, 1], f32, tag="sd")
    nc.scalar.activation(sd[:], mv[:, 1:2], AF.Sqrt, bias=g.epsT[:, 0:1], scale=1.0)
    nc.vector.reciprocal(sd[:], sd[:])
    nc.vector.tensor_scalar(
        out=htok[:, tt, :], in0=htok[:, tt, :],
        scalar1=mv[:, 0:1], scalar2=sd[:, 0:1],
        op0=ALU.subtract, op1=ALU.mult,
    )
    nc.vector.tensor_mul(htok[:, tt, :], htok[:, tt, :], lnw_b[:])
    nc.vector.tensor_add(htok[:, tt, :], htok[:, tt, :], lnb_b[:])


def _layer(g, l, hT):
    nc = g.nc
    par = g.biasp.tile([P, PARAM_COLS], f32, tag="par")
    nc.sync.dma_start(out=par[:], in_=g.params[l])
    bq = par[:, 0:6]
    bk = par[:, 6:12]
    bo = par[:, 12:18]
    bf2 = par[:, 18:24]
    l1w, l1b = par[:, 24:30], par[:, 30:36]
    l2w, l2b = par[:, 36:42], par[:, 42:48]
    bf1 = par[:, 48:72]

    # ---- V projection (token-major, f32r, ones col; bv folded into bo).
    # PE-heavy with no Act work: covers the previous layer's LN2 tail and
    # lets the Act engine run ahead on attention exps later. ----
    wv = g.wvp.tile([P, KC * D], f16, tag="wv")
    nc.sync.dma_start(out=wv[:], in_=g.WvT[l])
    v = g.vp.tile([P, H, NT, HD + 1], f32r, tag="v")
    nc.vector.tensor_copy(v[:, :, :, HD:HD + 1],
                          g.onesr[:].to_broadcast((P, H, NT, 1)))
    for tt in range(NT):
        for (cs0, cl) in ((0, 512), (512, 256)):
            ps = g.pp.tile([P, 512], f32, tag="pp")
            for k in range(KC):
                nc.tensor.matmul(
                    ps[:, :cl], lhsT=hT[:, k, tt * P:(tt + 1) * P],
                    rhs=wv[:, k * D + cs0: k * D + cs0 + cl],
                    start=(k == 0), stop=(k == KC - 1))
            nc.scalar.copy(
                v[:, cs0 // HD:(cs0 + cl) // HD, tt, 0:HD],
                ps[:, :cl].rearrange("p (h d) -> p h d", d=HD))

    # ---- fused K/Q projection + attention per feature block ----
    aT = g.act.tile([P, KC, N], f16, tag="act")
    for mcb in range(KC):
        wk = g.wp.tile([P, D], f16, tag="w")
        nc.sync.dma_start(out=wk[:], in_=g.WkT[l, mcb])
        kblk = g.qmp.tile([P, N], f16, tag="qm", name=f"kblk{mcb}")
        wq = g.wp.tile([P, D], f16, tag="w")
        nc.sync.dma_start(out=wq[:], in_=g.WqT[l, mcb])
        qm = g.qmp.tile([P, N], f16, tag="qm", name=f"qm{mcb}")
        for c in range(2):
            cs = slice(c * 512, (c + 1) * 512)
            ps = g.pp.tile([P, 512], f32, tag="pp")
            for k in range(KC):
                nc.tensor.matmul(
                    ps[:], lhsT=wk[:, k * P:(k + 1) * P], rhs=hT[:, k, cs],
                    start=(k == 0), stop=(k == KC - 1))
            nc.vector.tensor_scalar(
                out=kblk[:, cs], in0=ps[:], scalar1=bk[:, mcb:mcb + 1],
                scalar2=None, op0=ALU.add)
            ps = g.pp.tile([P, 512], f32, tag="pp")
            for k in range(KC):
                nc.tensor.matmul(
                    ps[:], lhsT=wq[:, k * P:(k + 1) * P], rhs=hT[:, k, cs],
                    start=(k == 0), stop=(k == KC - 1))
            nc.vector.tensor_scalar(
                out=qm[:, cs], in0=ps[:], scalar1=bq[:, mcb:mcb + 1],
                scalar2=None, op0=ALU.add)
        for hh in range(2):
            for s in range(2):
                _attn(g, mcb, hh, s, kblk, qm, v, aT)

    # ---- O projection + residual (c-outer); LN per chunk is pipelined ----
    x = g.act.tile([P, KC, N], f16, tag="act")
    for c in range(2):
        cs = slice(c * 512, (c + 1) * 512)
        for m in range(KC):
            wmb = g.wp.tile([P, D], f16, tag="w")
            nc.sync.dma_start(out=wmb[:], in_=g.WoT[l, m])
            ps = g.pp.tile([P, 512], f32, tag="pp")
            for k in range(KC):
                nc.tensor.matmul(
                    ps[:], lhsT=wmb[:, k * P:(k + 1) * P], rhs=aT[:, k, cs],
                    start=(k == 0), stop=(k == KC - 1))
            nc.scalar.activation(
                x[:, m, cs], ps[:], AF.Identity, bias=bo[:, m:m + 1], scale=1.0)
            nc.vector.tensor_add(x[:, m, cs], x[:, m, cs], hT[:, m, cs])
        _ln_chunk(g, x, l1w, l1b, c)
    h1 = x

    # ---- FFN (c-outer so chunk-0 work hides the chunk-1 LN1) ----
    ffT = g.ffp.tile([P, FC, N], f16, tag="ff")
    for c in range(2):
        cs = slice(c * 512, (c + 1) * 512)
        for m in range(FC):
            wmb = g.wp.tile([P, D], f16, tag="w")
            nc.sync.dma_start(out=wmb[:], in_=g.Wf1T[l, m])
            ps = g.pp.tile([P, 512], f32, tag="pp")
            for k in range(KC):
                nc.tensor.matmul(
                    ps[:], lhsT=wmb[:, k * P:(k + 1) * P], rhs=h1[:, k, cs],
                    start=(k == 0), stop=(k == KC - 1))
            nc.scalar.activation(
                ffT[:, m, cs], ps[:], g.act_fn, bias=bf1[:, m:m + 1], scale=1.0)
    x2 = g.act.tile([P, KC, N], f16, tag="act")
    for c in range(2):
        cs = slice(c * 512, (c + 1) * 512)
        for m in range(KC):
            w2 = g.w2p.tile([P, F], f16, tag="w2")
            nc.sync.dma_start(out=w2[:], in_=g.Wf2T[l, m])
            ps = g.pp.tile([P, 512], f32, tag="pp")
            for k in range(FC):
                nc.tensor.matmul(
                    ps[:], lhsT=w2[:, k * P:(k + 1) * P], rhs=ffT[:, k, cs],
                    start=(k == 0), stop=(k == FC - 1))
            nc.scalar.activation(
                x2[:, m, cs], ps[:], AF.Identity, bias=bf2[:, m:m + 1], scale=1.0)
            nc.vector.tensor_add(x2[:, m, cs], x2[:, m, cs], h1[:, m, cs])
        _ln_chunk(g, x2, l2w, l2b, c)
    return x2


def _attn(g, mcb, hh, s, kblk, qm, v, aT):
    nc = g.nc
    h = 2 * mcb + hh
    et = []
    for ck in range(4):
        sc = g.scp.tile([P, S], f32, tag="sc")
        nc.tensor.matmul(
            sc[:],
            lhsT=kblk[hh * HD:(hh + 1) * HD, s * S + ck * P:s * S + (ck + 1) * P],
            rhs=qm[hh * HD:(hh + 1) * HD, s * S:(s + 1) * S],
            start=True, stop=True)
        e = g.expp.tile([P, S], f32r, tag="exp")
        nc.scalar.activation(e[:], sc[:], AF.Exp, scale=0.125)
        et.append(e)
    av = g.avp.tile([HD + 1, S], f32, tag="av")
    for ck in range(4):
        nc.tensor.matmul(
            av[:], lhsT=v[:, h, s * 4 + ck, :], rhs=et[ck][:],
            start=(ck == 0), stop=(ck == 3))
    dinv = g.dvp.tile([1, S], f32, tag="dv", name="dinv")
    nc.vector.reciprocal(dinv[:], av[HD:HD + 1, :])
    dib = g.dvp.tile([HD, S], f32, tag="dv")
    nc.gpsimd.partition_broadcast(dib[:], dinv[:])
    nc.vector.tensor_tensor(
        out=aT[hh * HD:(hh + 1) * HD, mcb, s * S:(s + 1) * S],
        in0=av[0:HD, :], in1=dib[:], op=ALU.mult)


def _ln_chunk(g, x, w, b, c):
    """In-place LayerNorm of feature-major fp16 x for token chunk c."""
    nc = g.nc
    cs = slice(c * 512, (c + 1) * 512)
    rows = g.rowsp.tile([1, 4, 512], f32, tag="rows", name=f"rows{c}")
    mean, msq = rows[:, 0, :], rows[:, 1, :]
    A, B = rows[:, 2, :], rows[:, 3, :]
    pS = g.scp.tile([1, 512], f32, tag="sc", name=f"pS{c}")
    for k in range(KC):
        nc.tensor.matmul(pS[:], lhsT=g.ones[:], rhs=x[:, k, cs],
                         start=(k == 0), stop=(k == KC - 1))
    nc.vector.tensor_scalar(
        out=mean, in0=pS[:], scalar1=1.0 / D, scalar2=None, op0=ALU.mult)
    pQ = g.scp.tile([1, 512], f32, tag="sc", name=f"pQ{c}")
    for k in range(KC):
        sq = g.sqp.tile([P, 512], f16, tag="sq")
        nc.vector.tensor_mul(sq[:], x[:, k, cs], x[:, k, cs])
        nc.tensor.matmul(pQ[:], lhsT=g.ones[:], rhs=sq[:],
                         start=(k == 0), stop=(k == KC - 1))
    nc.vector.tensor_scalar(
        out=msq, in0=pQ[:], scalar1=1.0 / D, scalar2=None, op0=ALU.mult)
    nc.vector.tensor_mul(A, mean, mean)
    nc.vector.tensor_tensor(out=A, in0=msq, in1=A, op=ALU.subtract)
    nc.scalar.activation(A, A, AF.Sqrt, bias=g.epsT[0:1, 0:1], scale=1.0)
    nc.vector.reciprocal(A, A)
    nc.vector.tensor_mul(B, mean, A)
    # fp16 broadcast tiles keep the apply ops in the DVE 2x (16-bit) mode
    A16 = g.bb.tile([1, 512], f16, tag="bb", name=f"A16{c}")
    B16 = g.bb.tile([1, 512], f16, tag="bb", name=f"B16{c}")
    nc.vector.tensor_copy(A16[:], A)
    nc.vector.tensor_copy(B16[:], B)
    Ab = g.bb.tile([P, 512], f16, tag="bb")
    Bb = g.bb.tile([P, 512], f16, tag="bb")
    nc.gpsimd.partition_broadcast(Ab[:], A16[:])
    nc.gpsimd.partition_broadcast(Bb[:], B16[:])
    for k in range(KC):
        nc.vector.tensor_mul(x[:, k, cs], x[:, k, cs], Ab[:])
        nc.vector.tensor_tensor(out=x[:, k, cs], in0=x[:, k, cs], in1=Bb[:],
                                op=ALU.subtract)
        nc.vector.tensor_scalar(
            out=x[:, k, cs], in0=x[:, k, cs],
            scalar1=w[:, k:k + 1], scalar2=b[:, k:k + 1],
            op0=ALU.mult, op1=ALU.add)


# ======================= host-side prep / sharding =======================


def _r6(a, L, nchunk):
    # [L, D_or_F] -> [L, P, nchunk] with feature f = k*128 + p
    Ld = np.asarray(a)[:L]
    return Ld.reshape(L, nchunk, P).transpose(0, 2, 1).astype(np.float32)


def _blkT(W, L):
    # [L, Din, Dout] -> [L, Dout/P, P, Din]; [l, m, p, k*P+j] = W[l, k*P+p, m*P+j]
    Din, Dout = W.shape[1], W.shape[2]
    kc, mc = Din // P, Dout // P
    return np.ascontiguousarray(
        W.reshape(L, kc, P, mc, P).transpose(0, 3, 2, 1, 4).reshape(L, mc, P, Din)
    ).astype(np.float16)


def _rhsT(W, L):
    # [L, Din, Dout] -> [L, P, (Din/P)*Dout]; [l, p, k*Dout+n] = W[l, k*P+p, n]
    Din, Dout = W.shape[1], W.shape[2]
    kc = Din // P
    return np.ascontiguousarray(
        W.reshape(L, kc, P, Dout).transpose(0, 2, 1, 3).reshape(L, P, kc * Dout)
    ).astype(np.float16)


def prep_shared(inputs, L=12):
    f = lambda x: np.ascontiguousarray(np.asarray(x, dtype=np.float32))
    Wq = np.asarray(inputs["Wq"], np.float32)[:L]
    Wk = np.asarray(inputs["Wk"], np.float32)[:L]
    Wv = np.asarray(inputs["Wv"], np.float32)[:L]
    Wo = np.asarray(inputs["Wo"], np.float32)[:L]
    Wf1 = np.asarray(inputs["Wf1"], np.float32)[:L]
    Wf2 = np.asarray(inputs["Wf2"], np.float32)[:L]
    bv = np.asarray(inputs["bv"], np.float64)[:L]
    bo = np.asarray(inputs["bo"], np.float64)[:L]
    # fold the V bias through the O projection: o = Wo^T(A(v+bv)) + bo
    #   = Wo^T(Av) + (bo + Wo^T bv) since rows of A sum to 1 post-softmax
    bo_eff = bo + np.einsum("ld,ldo->lo", bv, Wo.astype(np.float64))
    params = np.concatenate([
        _r6(inputs["bq"], L, KC), _r6(inputs["bk"], L, KC),
        _r6(bo_eff.astype(np.float32), L, KC), _r6(inputs["bf2"], L, KC),
        _r6(inputs["ln1_w"], L, KC), _r6(inputs["ln1_b"], L, KC),
        _r6(inputs["ln2_w"], L, KC), _r6(inputs["ln2_b"], L, KC),
        _r6(inputs["bf1"], L, FC),
    ], axis=2)
    w = {
        "word_emb": np.ascontiguousarray(
            np.asarray(inputs["word_emb"], np.float32).astype(np.float16)),
        "pos_type": np.ascontiguousarray(
            (np.asarray(inputs["pos_emb"], np.float32)[:S] +
             np.asarray(inputs["type_emb"], np.float32)[0][None, :]
             ).astype(np.float16)),
        "emb_w": f(inputs["emb_ln_w"]).reshape(1, D),
        "emb_b": f(inputs["emb_ln_b"]).reshape(1, D),
        "WqT": _blkT(Wq, L), "WkT": _blkT(Wk, L), "WoT": _blkT(Wo, L),
        "WvT": _rhsT(Wv, L),
        "Wf1T": _blkT(Wf1, L), "Wf2T": _blkT(Wf2, L),
        "params": np.ascontiguousarray(params),
    }
    return w


def core_ids_input(input_ids, core):
    return np.ascontiguousarray(
        np.asarray(input_ids)[2 * core:2 * core + 2].reshape(N, 1)).astype(np.int32)


def assemble_output(out_fm):
    # [KC, P, N] feature-major -> [2, S, D] token-major
    return np.ascontiguousarray(out_fm.reshape(D, N).T).reshape(2, S, D)


_NC_CACHE = {}


def kernel(**inputs):
    from concourse.bass_utils import run_bass_kernel_spmd

    am = np.asarray(inputs["attention_mask"])
    assert (am == 1).all(), "kernel specialized for all-ones attention_mask"

    if "nc" not in _NC_CACHE:
        _NC_CACHE["nc"] = build_nc(L=12)
    nc = _NC_CACHE["nc"]

    prep_key = tuple(id(np.asarray(inputs[k])) for k in
                     ("Wq", "Wk", "Wv", "Wo", "Wf1", "Wf2", "word_emb"))
    if _NC_CACHE.get("prep_key") != prep_key:
        _NC_CACHE["shared"] = prep_shared(inputs, L=12)
        _NC_CACHE["prep_key"] = prep_key
    shared = _NC_CACHE["shared"]

    in_maps = []
    for core in range(8):
        m = dict(shared)
        m["ids"] = core_ids_input(inputs["input_ids"], core)
        in_maps.append(m)

    res = run_bass_kernel_spmd(nc, in_maps, list(range(8)), trace=False)
    out = np.concatenate(
        [assemble_output(res.results[c]["out_fm"]) for c in range(8)], axis=0)
    return out.astype(np.float32)


# revision 52
# speedup vs baseline: 1.1539x; 1.1539x over previous
"""Trainium2 Bass kernel for a 12-layer BERT encoder forward pass.

Strategy: data-parallel over the batch across 8 NeuronCores (2 sequences
each), no collectives. Activations are kept feature-major on-chip in fp16;
matmul weights are fp16 and host-pretransposed into SBUF-ready block layout
so every weight is DMA'd exactly once per layer with fully contiguous
descriptors. Attention uses a transposed-scores layout with a ones-column
appended to V so the softmax denominator falls out of the AV matmul
(exp-values and V are f32r to keep exp in f32 range at full PE speed).
The V bias is folded into the output-projection bias on the host.
PSUM accumulation, softmax and LayerNorm statistics stay f32.
kernel(**inputs) takes the full inputs and returns the full [16,512,768]
f32 output.
"""
import sys
for _p in ('/opt/trn_rl_repo', '/root/.axon_site/_ro/trn_rl_repo'):
    if _p not in sys.path:
        sys.path.append(_p)
import numpy as np
from contextlib import ExitStack

import concourse.bass as bass
from concourse import bacc
import concourse.mybir as mybir
import concourse.tile as tile
from concourse.masks import make_identity
from concourse import tile_utils

# allow using the full usable SBUF (stale default is 192KB/partition)
tile_utils.max_sbuf_usage = 208 * 1024

f32 = mybir.dt.float32
f32r = mybir.dt.float32r
f16 = mybir.dt.float16
f8 = mybir.dt.float8e4
i32 = mybir.dt.int32
AF = mybir.ActivationFunctionType
ALU = mybir.AluOpType
DR = mybir.MatmulPerfMode.DoubleRow

# fp8(e4m3) + DoubleRow for the FFN matmuls: 2 contraction tiles per
# instruction at 0.5 cycles/row = 4x PE throughput vs fp16 there
FFN_FP8 = False

P = 128
D = 768
KC = 6          # D / P
H = 12
HD = 64         # head dim
F = 3072
FC = 24         # F / P
S = 512
N = 1024        # tokens per core (2 seqs)
NT = 8          # N / P
EPS = 1e-12

# params column layout: 8 blocks of KC cols + FC cols for bf1
_PC = {"bq": 0, "bk": 6, "bo": 12, "bf2": 18,
       "l1w": 24, "l1b": 30, "l2w": 36, "l2b": 42, "bf1": 48}
PARAM_COLS = 72


class Ctx:
    pass


def build_nc(L=12, use_f32r=True, gelu_sim=False, reps=1):
    g = Ctx()
    nc = bacc.Bacc("TRN2", num_devices=8, dynamic_dma_scratch_size=4096)
    g.nc = nc
    g.act_fn = AF.Tanh if gelu_sim else AF.Gelu

    # ---- DRAM inputs ----
    g.ids = nc.dram_tensor("ids", [N, 1], i32, kind="ExternalInput")
    g.word_emb = nc.dram_tensor("word_emb", [30522, D], f16, kind="ExternalInput")
    g.pos_type = nc.dram_tensor("pos_type", [S, D], f16, kind="ExternalInput")
    g.emb_w = nc.dram_tensor("emb_w", [1, D], f32, kind="ExternalInput")
    g.emb_b = nc.dram_tensor("emb_b", [1, D], f32, kind="ExternalInput")
    g.WqT = nc.dram_tensor("WqT", [L, KC, P, D], f16, kind="ExternalInput")
    g.WkT = nc.dram_tensor("WkT", [L, KC, P, D], f16, kind="ExternalInput")
    g.WvT = nc.dram_tensor("WvT", [L, P, KC * D], f16, kind="ExternalInput")
    g.WoT = nc.dram_tensor("WoT", [L, KC, P, D], f16, kind="ExternalInput")
    fdt = f8 if FFN_FP8 else f16
    g.Wf1T = nc.dram_tensor("Wf1T", [L, FC, P, D], fdt, kind="ExternalInput")
    g.Wf2T = nc.dram_tensor("Wf2T", [L, KC, P, F], fdt, kind="ExternalInput")
    g.params = nc.dram_tensor("params", [L, P, PARAM_COLS], f32, kind="ExternalInput")
    g.out_fm = nc.dram_tensor("out_fm", [KC, P, N], f16, kind="ExternalOutput")

    with TileContextPools(g) as g:
        if reps > 1:
            with g.tc.For_i(0, reps, 1):
                _emit(g, L)
        else:
            _emit(g, L)

    nc.finalize()
    return nc


class TileContextPools:
    def __init__(self, g):
        self.g = g

    def __enter__(self):
        g = self.g
        self.stack = ExitStack()
        tc = self.stack.enter_context(tile.TileContext(g.nc))
        ep = self.stack.enter_context
        g.tc = tc
        g.act = ep(tc.tile_pool(name="act", bufs=5))      # fp16 [P,KC,N] = 12KB
        g.ffp = ep(tc.tile_pool(name="ffp", bufs=1))      # ffT / htok
        g.h8p = ep(tc.tile_pool(name="h8p", bufs=1))      # h1 in fp8, 6KB
        g.vp = ep(tc.tile_pool(name="vp", bufs=1))        # v f32r 24.4KB
        g.wvp = ep(tc.tile_pool(name="wvp", bufs=2))      # Wv fp16 9KB
        g.wp = ep(tc.tile_pool(name="wp", bufs=4))        # weight blocks fp16 1.5KB
        g.w2p = ep(tc.tile_pool(name="w2p", bufs=2))      # Wf2 blocks fp16 6KB
        g.qmp = ep(tc.tile_pool(name="qmp", bufs=3))      # qm/kblk fp16 2KB
        g.sqp = ep(tc.tile_pool(name="sqp", bufs=2))      # LN x^2 fp16 1KB
        g.expp = ep(tc.tile_pool(name="expp", bufs=5))    # exp f32r 2KB
        g.dvp = ep(tc.tile_pool(name="dvp", bufs=2))      # dinv bcast f32
        g.bb = ep(tc.tile_pool(name="bb", bufs=8))        # LN A/B bcast f16 1KB
        g.rowsp = ep(tc.tile_pool(name="rows", bufs=1))
        g.rowp = ep(tc.tile_pool(name="rowp", bufs=1))
        g.singles = ep(tc.tile_pool(name="singles", bufs=1))
        g.small = ep(tc.tile_pool(name="small", bufs=4))
        g.biasp = ep(tc.tile_pool(name="bias", bufs=2))
        g.pp = ep(tc.tile_pool(name="pp", bufs=4, space="PSUM"))
        g.scp = ep(tc.tile_pool(name="scp", bufs=2, space="PSUM"))
        g.avp = ep(tc.tile_pool(name="avp", bufs=2, space="PSUM"))
        return g

    def __exit__(self, *a):
        return self.stack.__exit__(*a)


def _emit(g, L):
    nc = g.nc
    g.ident = g.singles.tile([P, P], f32, tag="ident")
    make_identity(nc, g.ident[:])
    g.ones = g.singles.tile([P, 1], f16, tag="ones")
    nc.vector.memset(g.ones[:], 1.0)
    g.epsT = g.singles.tile([P, 1], f32, tag="eps")
    nc.vector.memset(g.epsT[:], EPS)
    g.onesr = g.singles.tile([P, 1], f32, tag="onesr")
    nc.vector.memset(g.onesr[:], 1.0)

    hT = _embedding(g)
    for l in range(L):
        hT = _layer(g, l, hT)
    for k in range(KC):
        nc.sync.dma_start(out=g.out_fm[k], in_=hT[:, k, :])


def _embedding(g):
    nc = g.nc
    lnw_b = g.wvp.tile([P, D], f32, tag="wv", name="emb_lnw")
    lnb_b = g.wvp.tile([P, D], f32, tag="wv", name="emb_lnb")
    embwb = g.rowp.tile([1, 2, D], f32, tag="row")
    nc.sync.dma_start(out=embwb[:, 0, :], in_=g.emb_w[:])
    nc.gpsimd.partition_broadcast(lnw_b[:], embwb[:, 0, :])
    nc.sync.dma_start(out=embwb[:, 1, :], in_=g.emb_b[:])
    nc.gpsimd.partition_broadcast(lnb_b[:], embwb[:, 1, :])

    htok = g.ffp.tile([P, NT, D], f32, tag="ff")
    hT = g.act.tile([P, KC, N], f16, tag="act")
    for tt in range(NT):
        _embed_tile(g, htok, tt, lnw_b, lnb_b)
        for k in range(KC):
            ps = g.pp.tile([P, 512], f32, tag="pp")
            nc.tensor.transpose(ps[:, 0:P], htok[:, tt, k * P:(k + 1) * P], g.ident[:])
            nc.vector.tensor_copy(hT[:, k, tt * P:(tt + 1) * P], ps[:, 0:P])
    return hT


def _embed_tile(g, htok, tt, lnw_b, lnb_b):
    nc = g.nc
    idx = g.small.tile([P, 1], i32, tag="idx")
    nc.sync.dma_start(out=idx[:], in_=g.ids[tt * P:(tt + 1) * P, :])
    gt = g.wp.tile([P, D], f16, tag="w")
    nc.gpsimd.indirect_dma_start(
        out=gt[:], out_offset=None, in_=g.word_emb[:],
        in_offset=bass.IndirectOffsetOnAxis(ap=idx[:, :1], axis=0),
    )
    pt = g.wp.tile([P, D], f16, tag="w")
    nc.sync.dma_start(out=pt[:], in_=g.pos_type[(tt % 4) * P:(tt % 4 + 1) * P, :])
    nc.vector.tensor_add(htok[:, tt, :], gt[:], pt[:])
    xr = htok[:, tt, :].rearrange("p (s f) -> p s f", f=256)
    stats = g.small.tile([P, 3, 6], f32, tag="bnst")
    for sgi in range(3):
        nc.vector.bn_stats(out=stats[:, sgi, :], in_=xr[:, sgi, :])
    mv = g.small.tile([P, 2], f32, tag="bnmv")
    nc.vector.bn_aggr(out=mv[:], in_=stats[:])
    sd = g.small.tile([P, 1], f32, tag="sd")
    nc.scalar.activation(sd[:], mv[:, 1:2], AF.Sqrt, bias=g.epsT[:, 0:1], scale=1.0)
    nc.vector.reciprocal(sd[:], sd[:])
    nc.vector.tensor_scalar(
        out=htok[:, tt, :], in0=htok[:, tt, :],
        scalar1=mv[:, 0:1], scalar2=sd[:, 0:1],
        op0=ALU.subtract, op1=ALU.mult,
    )
    nc.gpsimd.tensor_mul(htok[:, tt, :], htok[:, tt, :], lnw_b[:])
    nc.gpsimd.tensor_add(htok[:, tt, :], htok[:, tt, :], lnb_b[:])


def _layer(g, l, hT):
    nc = g.nc
    par = g.biasp.tile([P, PARAM_COLS], f32, tag="par")
    nc.sync.dma_start(out=par[:], in_=g.params[l])
    bq = par[:, 0:6]
    bk = par[:, 6:12]
    bo = par[:, 12:18]
    bf2 = par[:, 18:24]
    l1w, l1b = par[:, 24:30], par[:, 30:36]
    l2w, l2b = par[:, 36:42], par[:, 42:48]
    bf1 = par[:, 48:72]

    # ---- V projection (token-major, f32r, ones col; bv folded into bo).
    # PE-heavy with no Act work: covers the previous layer's LN2 tail and
    # lets the Act engine run ahead on attention exps later. ----
    wv = g.wvp.tile([P, KC * D], f16, tag="wv")
    nc.sync.dma_start(out=wv[:], in_=g.WvT[l])
    v = g.vp.tile([P, H, NT, HD + 1], f32r, tag="v")
    nc.vector.tensor_copy(v[:, :, :, HD:HD + 1],
                          g.onesr[:].to_broadcast((P, H, NT, 1)))
    for tt in range(NT):
        for (cs0, cl) in ((0, 512), (512, 256)):
            ps = g.pp.tile([P, 512], f32, tag="pp")
            for k in range(KC):
                nc.tensor.matmul(
                    ps[:, :cl], lhsT=hT[:, k, tt * P:(tt + 1) * P],
                    rhs=wv[:, k * D + cs0: k * D + cs0 + cl],
                    start=(k == 0), stop=(k == KC - 1))
            nc.scalar.copy(
                v[:, cs0 // HD:(cs0 + cl) // HD, tt, 0:HD],
                ps[:, :cl].rearrange("p (h d) -> p h d", d=HD))

    # ---- fused K/Q projection + attention per feature block ----
    aT = g.act.tile([P, KC, N], f16, tag="act")
    for mcb in range(KC):
        wk = g.wp.tile([P, D], f16, tag="w")
        nc.sync.dma_start(out=wk[:], in_=g.WkT[l, mcb])
        kblk = g.qmp.tile([P, N], f16, tag="qm", name=f"kblk{mcb}")
        wq = g.wp.tile([P, D], f16, tag="w")
        nc.sync.dma_start(out=wq[:], in_=g.WqT[l, mcb])
        qm = g.qmp.tile([P, N], f16, tag="qm", name=f"qm{mcb}")
        for c in range(2):
            cs = slice(c * 512, (c + 1) * 512)
            ps = g.pp.tile([P, 512], f32, tag="pp")
            for k in range(KC):
                nc.tensor.matmul(
                    ps[:], lhsT=wk[:, k * P:(k + 1) * P], rhs=hT[:, k, cs],
                    start=(k == 0), stop=(k == KC - 1))
            nc.vector.tensor_scalar(
                out=kblk[:, cs], in0=ps[:], scalar1=bk[:, mcb:mcb + 1],
                scalar2=None, op0=ALU.add)
            ps = g.pp.tile([P, 512], f32, tag="pp")
            for k in range(KC):
                nc.tensor.matmul(
                    ps[:], lhsT=wq[:, k * P:(k + 1) * P], rhs=hT[:, k, cs],
                    start=(k == 0), stop=(k == KC - 1))
            nc.vector.tensor_scalar(
                out=qm[:, cs], in0=ps[:], scalar1=bq[:, mcb:mcb + 1],
                scalar2=None, op0=ALU.add)
        for hh in range(2):
            for s in range(2):
                _attn(g, mcb, hh, s, kblk, qm, v, aT)

    # ---- O projection + residual (c-outer); LN per chunk is pipelined ----
    x = g.act.tile([P, KC, N], f16, tag="act")
    for c in range(2):
        cs = slice(c * 512, (c + 1) * 512)
        for m in range(KC):
            wmb = g.wp.tile([P, D], f16, tag="w")
            nc.sync.dma_start(out=wmb[:], in_=g.WoT[l, m])
            ps = g.pp.tile([P, 512], f32, tag="pp")
            for k in range(KC):
                nc.tensor.matmul(
                    ps[:], lhsT=wmb[:, k * P:(k + 1) * P], rhs=aT[:, k, cs],
                    start=(k == 0), stop=(k == KC - 1))
            nc.scalar.activation(
                x[:, m, cs], ps[:], AF.Identity, bias=bo[:, m:m + 1], scale=1.0)
            nc.vector.tensor_add(x[:, m, cs], x[:, m, cs], hT[:, m, cs])
        _ln_chunk(g, x, l1w, l1b, c)
    h1 = x

    # ---- FFN (c-outer so chunk-0 work hides the chunk-1 LN1) ----
    if FFN_FP8:
        h8 = g.h8p.tile([P, KC, N], f8, tag="h8")
        ffT = g.ffp.tile([P, FC, N], f8, tag="ff")
    else:
        ffT = g.ffp.tile([P, FC, N], f16, tag="ff")
    x2 = g.act.tile([P, KC, N], f16, tag="act")
    for c in range(2):
        cs = slice(c * 512, (c + 1) * 512)
        if FFN_FP8:
            for k in range(KC):
                nc.gpsimd.tensor_copy(h8[:, k, cs], h1[:, k, cs])
        for m in range(FC):
            if FFN_FP8:
                wmb = g.wp.tile([P, 3, 2, P], f8, tag="w", name=f"w1_{m}")
                nc.sync.dma_start(
                    out=wmb[:], in_=g.Wf1T[l, m].rearrange(
                        "p (a b j) -> p a b j", a=3, b=2))
                ps = g.pp.tile([P, 512], f32, tag="pp")
                for k2 in range(3):
                    nc.tensor.matmul(
                        ps[:], lhsT=wmb[:, k2], rhs=h8[:, 2 * k2:2 * k2 + 2, cs],
                        start=(k2 == 0), stop=(k2 == 2), perf_mode=DR)
            else:
                wmb = g.wp.tile([P, D], f16, tag="w")
                nc.sync.dma_start(out=wmb[:], in_=g.Wf1T[l, m])
                ps = g.pp.tile([P, 512], f32, tag="pp")
                for k in range(KC):
                    nc.tensor.matmul(
                        ps[:], lhsT=wmb[:, k * P:(k + 1) * P], rhs=h1[:, k, cs],
                        start=(k == 0), stop=(k == KC - 1))
            nc.scalar.activation(
                ffT[:, m, cs], ps[:], g.act_fn, bias=bf1[:, m:m + 1], scale=1.0)
        for m in range(KC):
            ps = g.pp.tile([P, 512], f32, tag="pp")
            if FFN_FP8:
                w2 = g.w2p.tile([P, 12, 2, P], f8, tag="w2", name=f"w2_{m}")
                nc.sync.dma_start(
                    out=w2[:], in_=g.Wf2T[l, m].rearrange(
                        "p (a b j) -> p a b j", a=12, b=2))
                for j in range(12):
                    nc.tensor.matmul(
                        ps[:], lhsT=w2[:, j], rhs=ffT[:, 2 * j:2 * j + 2, cs],
                        start=(j == 0), stop=(j == 11), perf_mode=DR)
            else:
                w2 = g.w2p.tile([P, F], f16, tag="w2")
                nc.sync.dma_start(out=w2[:], in_=g.Wf2T[l, m])
                for k in range(FC):
                    nc.tensor.matmul(
                        ps[:], lhsT=w2[:, k * P:(k + 1) * P], rhs=ffT[:, k, cs],
                        start=(k == 0), stop=(k == FC - 1))
            nc.scalar.activation(
                x2[:, m, cs], ps[:], AF.Identity, bias=bf2[:, m:m + 1], scale=1.0)
            nc.vector.tensor_add(x2[:, m, cs], x2[:, m, cs], h1[:, m, cs])
        _ln_chunk(g, x2, l2w, l2b, c)
    return x2


def _attn(g, mcb, hh, s, kblk, qm, v, aT):
    nc = g.nc
    h = 2 * mcb + hh
    et = []
    for ck in range(4):
        sc = g.scp.tile([P, S], f32, tag="sc")
        nc.tensor.matmul(
            sc[:],
            lhsT=kblk[hh * HD:(hh + 1) * HD, s * S + ck * P:s * S + (ck + 1) * P],
            rhs=qm[hh * HD:(hh + 1) * HD, s * S:(s + 1) * S],
            start=True, stop=True)
        e = g.expp.tile([P, S], f32r, tag="exp")
        nc.scalar.activation(e[:], sc[:], AF.Exp, scale=0.125)
        et.append(e)
    av = g.avp.tile([HD + 1, S], f32, tag="av")
    for ck in range(4):
        nc.tensor.matmul(
            av[:], lhsT=v[:, h, s * 4 + ck, :], rhs=et[ck][:],
            start=(ck == 0), stop=(ck == 3))
    dinv = g.dvp.tile([1, S], f32, tag="dv", name="dinv")
    nc.vector.reciprocal(dinv[:], av[HD:HD + 1, :])
    dib = g.dvp.tile([HD, S], f32, tag="dv")
    nc.gpsimd.partition_broadcast(dib[:], dinv[:])
    nc.vector.tensor_tensor(
        out=aT[hh * HD:(hh + 1) * HD, mcb, s * S:(s + 1) * S],
        in0=av[0:HD, :], in1=dib[:], op=ALU.mult)


def _ln_chunk(g, x, w, b, c):
    """In-place LayerNorm of feature-major fp16 x for token chunk c."""
    nc = g.nc
    cs = slice(c * 512, (c + 1) * 512)
    rows = g.rowsp.tile([1, 4, 512], f32, tag="rows", name=f"rows{c}")
    mean, msq = rows[:, 0, :], rows[:, 1, :]
    A, B = rows[:, 2, :], rows[:, 3, :]
    pS = g.scp.tile([1, 512], f32, tag="sc", name=f"pS{c}")
    for k in range(KC):
        nc.tensor.matmul(pS[:], lhsT=g.ones[:], rhs=x[:, k, cs],
                         start=(k == 0), stop=(k == KC - 1))
    nc.vector.tensor_scalar(
        out=mean, in0=pS[:], scalar1=1.0 / D, scalar2=None, op0=ALU.mult)
    pQ = g.scp.tile([1, 512], f32, tag="sc", name=f"pQ{c}")
    for k in range(KC):
        sq = g.sqp.tile([P, 512], f16, tag="sq")
        nc.vector.tensor_mul(sq[:], x[:, k, cs], x[:, k, cs])
        nc.tensor.matmul(pQ[:], lhsT=g.ones[:], rhs=sq[:],
                         start=(k == 0), stop=(k == KC - 1))
    nc.vector.tensor_scalar(
        out=msq, in0=pQ[:], scalar1=1.0 / D, scalar2=None, op0=ALU.mult)
    nc.vector.tensor_mul(A, mean, mean)
    nc.vector.tensor_tensor(out=A, in0=msq, in1=A, op=ALU.subtract)
    nc.scalar.activation(A, A, AF.Sqrt, bias=g.epsT[0:1, 0:1], scale=1.0)
    nc.vector.reciprocal(A, A)
    nc.vector.tensor_mul(B, mean, A)
    # fp16 broadcast tiles keep the apply ops in the DVE 2x (16-bit) mode
    A16 = g.bb.tile([1, 512], f16, tag="bb", name=f"A16{c}")
    B16 = g.bb.tile([1, 512], f16, tag="bb", name=f"B16{c}")
    nc.vector.tensor_copy(A16[:], A)
    nc.vector.tensor_copy(B16[:], B)
    Ab = g.bb.tile([P, 512], f16, tag="bb")
    Bb = g.bb.tile([P, 512], f16, tag="bb")
    nc.gpsimd.partition_broadcast(Ab[:], A16[:])
    nc.gpsimd.partition_broadcast(Bb[:], B16[:])
    for k in range(KC):
        nc.vector.tensor_mul(x[:, k, cs], x[:, k, cs], Ab[:])
        nc.vector.tensor_tensor(out=x[:, k, cs], in0=x[:, k, cs], in1=Bb[:],
                                op=ALU.subtract)
        nc.vector.tensor_scalar(
            out=x[:, k, cs], in0=x[:, k, cs],
            scalar1=w[:, k:k + 1], scalar2=b[:, k:k + 1],
            op0=ALU.mult, op1=ALU.add)


# ======================= host-side prep / sharding =======================


def _r6(a, L, nchunk):
    # [L, D_or_F] -> [L, P, nchunk] with feature f = k*128 + p
    Ld = np.asarray(a)[:L]
    return Ld.reshape(L, nchunk, P).transpose(0, 2, 1).astype(np.float32)


def _blkT(W, L, dtype=np.float16):
    # [L, Din, Dout] -> [L, Dout/P, P, Din]; [l, m, p, k*P+j] = W[l, k*P+p, m*P+j]
    Din, Dout = W.shape[1], W.shape[2]
    kc, mc = Din // P, Dout // P
    return np.ascontiguousarray(
        W.reshape(L, kc, P, mc, P).transpose(0, 3, 2, 1, 4).reshape(L, mc, P, Din)
    ).astype(dtype)


def _rhsT(W, L):
    # [L, Din, Dout] -> [L, P, (Din/P)*Dout]; [l, p, k*Dout+n] = W[l, k*P+p, n]
    Din, Dout = W.shape[1], W.shape[2]
    kc = Din // P
    return np.ascontiguousarray(
        W.reshape(L, kc, P, Dout).transpose(0, 2, 1, 3).reshape(L, P, kc * Dout)
    ).astype(np.float16)


def _fdt_np():
    return mybir.dt.np(f8) if FFN_FP8 else np.float16


def prep_shared(inputs, L=12):
    f = lambda x: np.ascontiguousarray(np.asarray(x, dtype=np.float32))
    Wq = np.asarray(inputs["Wq"], np.float32)[:L]
    Wk = np.asarray(inputs["Wk"], np.float32)[:L]
    Wv = np.asarray(inputs["Wv"], np.float32)[:L]
    Wo = np.asarray(inputs["Wo"], np.float32)[:L]
    Wf1 = np.asarray(inputs["Wf1"], np.float32)[:L]
    Wf2 = np.asarray(inputs["Wf2"], np.float32)[:L]
    bv = np.asarray(inputs["bv"], np.float64)[:L]
    bo = np.asarray(inputs["bo"], np.float64)[:L]
    # fold the V bias through the O projection: o = Wo^T(A(v+bv)) + bo
    #   = Wo^T(Av) + (bo + Wo^T bv) since rows of A sum to 1 post-softmax
    bo_eff = bo + np.einsum("ld,ldo->lo", bv, Wo.astype(np.float64))
    params = np.concatenate([
        _r6(inputs["bq"], L, KC), _r6(inputs["bk"], L, KC),
        _r6(bo_eff.astype(np.float32), L, KC), _r6(inputs["bf2"], L, KC),
        _r6(inputs["ln1_w"], L, KC), _r6(inputs["ln1_b"], L, KC),
        _r6(inputs["ln2_w"], L, KC), _r6(inputs["ln2_b"], L, KC),
        _r6(inputs["bf1"], L, FC),
    ], axis=2)
    w = {
        "word_emb": np.ascontiguousarray(
            np.asarray(inputs["word_emb"], np.float32).astype(np.float16)),
        "pos_type": np.ascontiguousarray(
            (np.asarray(inputs["pos_emb"], np.float32)[:S] +
             np.asarray(inputs["type_emb"], np.float32)[0][None, :]
             ).astype(np.float16)),
        "emb_w": f(inputs["emb_ln_w"]).reshape(1, D),
        "emb_b": f(inputs["emb_ln_b"]).reshape(1, D),
        "WqT": _blkT(Wq, L), "WkT": _blkT(Wk, L), "WoT": _blkT(Wo, L),
        "WvT": _rhsT(Wv, L),
        "Wf1T": _blkT(Wf1, L, _fdt_np()), "Wf2T": _blkT(Wf2, L, _fdt_np()),
        "params": np.ascontiguousarray(params),
    }
    return w


def core_ids_input(input_ids, core):
    return np.ascontiguousarray(
        np.asarray(input_ids)[2 * core:2 * core + 2].reshape(N, 1)).astype(np.int32)


def assemble_output(out_fm):
    # [KC, P, N] feature-major -> [2, S, D] token-major
    return np.ascontiguousarray(out_fm.reshape(D, N).T).reshape(2, S, D)


_NC_CACHE = {}


def kernel(**inputs):
    from concourse.bass_utils import run_bass_kernel_spmd

    am = np.asarray(inputs["attention_mask"])
    assert (am == 1).all(), "kernel specialized for all-ones attention_mask"

    if "nc" not in _NC_CACHE:
        _NC_CACHE["nc"] = build_nc(L=12)
    nc = _NC_CACHE["nc"]

    prep_key = tuple(id(np.asarray(inputs[k])) for k in
                     ("Wq", "Wk", "Wv", "Wo", "Wf1", "Wf2", "word_emb"))
    if _NC_CACHE.get("prep_key") != prep_key:
        _NC_CACHE["shared"] = prep_shared(inputs, L=12)
        _NC_CACHE["prep_key"] = prep_key
    shared = _NC_CACHE["shared"]

    in_maps = []
    for core in range(8):
        m = dict(shared)
        m["ids"] = core_ids_input(inputs["input_ids"], core)
        in_maps.append(m)

    res = run_bass_kernel_spmd(nc, in_maps, list(range(8)), trace=False)
    out = np.concatenate(
        [assemble_output(res.results[c]["out_fm"]) for c in range(8)], axis=0)
    return out.astype(np.float32)
